# revision 32
# baseline (speedup 1.0000x reference)
"""Gaussian-mixture attention (first decoder step) on 8 Trainium2 NeuronCores.

Data-parallel over the batch dim (64 -> 8 cores x 8). All math runs on device;
the host only shards/relayouts inputs and concatenates outputs.

Per-core pipeline:
  1. MLP: hiddenT = tanh(W1'^T @ h^T + b1)  (PE, contraction over H=1024)
     mlp = hiddenT^T @ W2 -> (8, 48)        (only first H rows of W1 matter:
     last context vector is exactly zero on the first step)
  2. coefficients per (b,k): mu = softplus(delta), Sigma = softplus(sigma),
     logW = log_softmax(w). Folded into ACT-friendly per-partition scalars
     (bias05 = 0.5 - mu, ninv2s = -1/(2 Sigma^2), c0 = logW - ln Sigma - .5*ln 2pi)
     and moved to the (b*16+k) partition layout via a tiny DRAM roundtrip.
  3. P[(b,k), t] = exp(c0 - (J_t - mu)^2 / (2 Sigma^2)) in two ACT passes over
     (128, 512) tiles (J from a one-time iota); sum over k via one PE matmul
     with a 0/1 block-selector (128, 8) -> S[b, t] in PSUM.
     log then clamp at -18: plain sum-exp is exact-equivalent to the
     reference's stable logsumexp because anything below -18 is clamped and
     values above -80 don't underflow.
  4. mask + softmax over t on (8, 4096) in place.
  5. energy chunks transposed on PE (interleaved with the score stage);
     context matvec ctx[b] = sum_t energy[b,t] * memory[b,t,:] accumulated
     in PSUM while 32 DMAs stream the memory shard (the roofline term).
     The 1/sum(energy) normalization is folded into the PSUM drain.

Precision: memory streams as fp16 by default (MEM_MODE env var overrides:
f32 / f32r / bf16). fp16 halves the HBM traffic and runs the PE at full
rate; measured output error is ~5e-4 scale-relative on ctx (vs ~8e-5 for
full f32, ~2e-4 for f32r) while the energy output stays at ~1e-5.
"""

import math
import os
from contextlib import ExitStack

import ml_dtypes
import numpy as np

import concourse.bacc as bacc
import concourse.bass as bass
import concourse.mybir as mybir
import concourse.tile as tile
from concourse.bass_utils import run_bass_kernel_spmd

AF = mybir.ActivationFunctionType
ALU = mybir.AluOpType
F32 = mybir.dt.float32
U8 = mybir.dt.uint8

B, TT, H, F, A, K = 64, 4096, 1024, 768, 128, 16
NCORES = 8
BL = B // NCORES          # 8 batches per core
NC512 = TT // 512         # 8 chunks for the score stage
NGRP = TT // 1024         # 4 memory tiles per batch (8 t-chunks of 128 each)

# context-matvec precision mode:
#   f32  -- exact fp32 matmuls (4 cycles/row on the PE)
#   f32r -- fp32 data in single-pass PE mode (1 cycle/row at N>=256)
#   bf16 -- memory downcast to bf16 on host (half DMA traffic, 1 cycle/row)
MEM_MODE = os.environ.get("NN_ATTN_MEM_MODE", "f16")
MEM_DT = {"bf16": mybir.dt.bfloat16,
          "f16": mybir.dt.float16,
          "f32r": mybir.dt.float32r,
          "f32": F32}[MEM_MODE]
# ~120 KB/partition of SBUF runway for the memory stream either way
MEM_BUFS = 10 if MEM_DT in (mybir.dt.bfloat16, mybir.dt.float16) else 5

LOG2PI_HALF = 0.5 * math.log(2.0 * math.pi)


def _build_bass():
    nc = bacc.Bacc(
        "TRN2", target_bir_lowering=False, debug=False, num_devices=NCORES
    )

    hT_d = nc.dram_tensor("hT", [128, 8 * BL], F32, kind="ExternalInput").ap()
    mem_d = nc.dram_tensor("mem", [BL, TT, F], MEM_DT, kind="ExternalInput").ap()
    mask_d = nc.dram_tensor("mask", [BL, TT], U8, kind="ExternalInput").ap()
    w1_d = nc.dram_tensor("w1", [128, H], F32, kind="ExternalInput").ap()
    b1_d = nc.dram_tensor("b1", [128, 1], F32, kind="ExternalInput").ap()
    w2_d = nc.dram_tensor("w2", [128, 3 * K], F32, kind="ExternalInput").ap()
    esel_d = nc.dram_tensor("esel", [128, BL], F32, kind="ExternalInput").ap()
    eselT_d = nc.dram_tensor("eselT", [BL, 128], F32, kind="ExternalInput").ap()
    dmask_d = nc.dram_tensor("dmask", [128, 3 * K], F32, kind="ExternalInput").ap()

    ctx_d = nc.dram_tensor("ctx", [BL, F], F32, kind="ExternalOutput").ap()
    energy_d = nc.dram_tensor("energy", [BL, TT], F32, kind="ExternalOutput").ap()

    with tile.TileContext(nc) as tc, ExitStack() as ctx:
        _kernel(ctx, tc, hT_d, mem_d, mask_d, w1_d, b1_d, w2_d, esel_d,
                eselT_d, dmask_d, ctx_d, energy_d)
    nc.compile()
    return nc


def _kernel(ctx, tc, hT_d, mem_d, mask_d, w1_d, b1_d, w2_d, esel_d,
            eselT_d, dmask_d, ctx_d, energy_d):
    nc = tc.nc

    consts = ctx.enter_context(tc.tile_pool(name="consts", bufs=1))
    small = ctx.enter_context(tc.tile_pool(name="small", bufs=1))
    ppool = ctx.enter_context(tc.tile_pool(name="ppool", bufs=3))
    mem_pool = ctx.enter_context(tc.tile_pool(name="mem_pool", bufs=MEM_BUFS))
    dram = ctx.enter_context(tc.tile_pool(name="dram", bufs=1, space="DRAM"))

    # PSUM bank budget (8 banks): mlp 2 (two tags x 1) + sp 3 + tr 1 +
    # ctx 2 (one (1,768) tile spanning 2 banks)
    ps_mlp = ctx.enter_context(tc.tile_pool(name="ps_mlp", bufs=1, space="PSUM"))
    ps_s = ctx.enter_context(tc.tile_pool(name="ps_s", bufs=2, space="PSUM"))
    ps_tr = ctx.enter_context(tc.tile_pool(name="ps_tr", bufs=1, space="PSUM"))
    ps_ctx = ctx.enter_context(tc.tile_pool(name="ps_ctx", bufs=2, space="PSUM"))

    # ---- constant / small input loads -------------------------------------
    w1_sb = consts.tile([128, H], F32)
    nc.scalar.dma_start(w1_sb[:], w1_d[:])
    w2_sb = consts.tile([128, 3 * K], F32)
    nc.scalar.dma_start(w2_sb[:], w2_d[:])
    b1_sb = consts.tile([128, 1], F32)
    nc.scalar.dma_start(b1_sb[:], b1_d[:])
    esel_sb = consts.tile([128, BL], F32)
    nc.scalar.dma_start(esel_sb[:], esel_d[:])
    eselT_sb = consts.tile([BL, 128], F32)
    nc.scalar.dma_start(eselT_sb[:], eselT_d[:])
    dmask_sb = consts.tile([128, 3 * K], F32)
    nc.scalar.dma_start(dmask_sb[:], dmask_d[:])
    hT_sb = consts.tile([128, 8 * BL], F32)
    nc.scalar.dma_start(hT_sb[:], hT_d[:])


    # J tile: value t at [p, t] for every partition (one-time iota, f32
    # exact; generated during the MLP head, off the critical path)
    iota_sb = consts.tile([128, TT], F32)
    nc.gpsimd.iota(iota_sb[:], pattern=[[1, TT]], base=0, channel_multiplier=0,
                   allow_small_or_imprecise_dtypes=True)

    # ---- stage 1: MLP ------------------------------------------------------
    mm1 = ps_mlp.tile([128, BL], F32, tag="mm1")
    for c in range(H // 128):
        nc.tensor.matmul(mm1[:], w1_sb[:, bass.ts(c, 128)],
                         hT_sb[:, bass.ts(c, BL)],
                         start=(c == 0), stop=(c == H // 128 - 1))
    # tanh(y) = 1 - 2/(exp(2y) + 1), y = mm1 + b1 (b1_sb holds 2*b1 so the
    # bias folds into the Exp's scale/bias form). Keeps every activation in
    # the single natural_log_exp table (Tanh would force a table reload).
    e2y = small.tile([128, BL], F32)
    nc.scalar.activation(e2y[:], mm1[:], AF.Exp, bias=b1_sb[:, 0:1], scale=2.0)
    nc.vector.tensor_scalar_add(e2y[:], e2y[:], 1.0)
    hidT = small.tile([128, BL], F32)
    nc.vector.reciprocal(hidT[:], e2y[:])
    nc.vector.tensor_scalar(hidT[:], hidT[:], -2.0, 1.0,
                            op0=ALU.mult, op1=ALU.add)

    mm2 = ps_mlp.tile([BL, 3 * K], F32, tag="mm1")
    nc.tensor.matmul(mm2[:], hidT[:], w2_sb[:], start=True, stop=True)
    mlp_sb = small.tile([BL, 3 * K], F32)
    nc.vector.tensor_copy(mlp_sb[:], mm2[:])

    # ---- stage 2: per-(b,k) coefficients in (BL, K) layout ----------------
    w_ = mlp_sb[:, 0:K]
    d_ = mlp_sb[:, K:2 * K]
    s_ = mlp_sb[:, 2 * K:3 * K]

    # softplus(x) = ln(1 + exp(x)) (Softplus has no ACT table on this build;
    # |x| <~ 5 here so the direct form is accurate in f32). d_ and s_ are
    # adjacent in mlp_sb, so both softplus chains run as one (8, 32) chain.
    ds = mlp_sb[:, K:3 * K]
    musig = small.tile([BL, 2 * K], F32)
    nc.scalar.activation(musig[:], ds, AF.Exp)
    nc.vector.tensor_scalar_add(musig[:], musig[:], 1.0)
    nc.scalar.activation(musig[:], musig[:], AF.Ln)
    mu = musig[:, 0:K]
    sig = musig[:, K:2 * K]

    coefpack = small.tile([BL, 3 * K], F32)
    # -1 / (2 sigma^2)
    sig2 = small.tile([BL, K], F32)
    nc.vector.tensor_mul(sig2[:], sig, sig)
    nc.vector.tensor_scalar_mul(sig2[:], sig2[:], 2.0)
    ninv2s = coefpack[:, K:2 * K]
    nc.vector.reciprocal(ninv2s, sig2[:])
    nc.vector.tensor_scalar_mul(ninv2s, ninv2s, -1.0)

    logsig = small.tile([BL, K], F32)
    nc.scalar.activation(logsig[:], sig, AF.Ln)

    # log-softmax of w over k (free dim)
    nwmax = small.tile([BL, 1], F32)
    nc.vector.tensor_reduce(nwmax[:], w_, axis=mybir.AxisListType.X,
                            op=ALU.max, negate=True)
    expw = small.tile([BL, K], F32)
    wsum = small.tile([BL, 1], F32)
    nc.scalar.activation(expw[:], w_, AF.Exp, bias=nwmax[:, 0:1],
                         accum_out=wsum[:, 0:1])
    lnwsum = small.tile([BL, 1], F32)
    nc.scalar.activation(lnwsum[:], wsum[:], AF.Ln)
    # soff = -max - ln(sum)
    soff = small.tile([BL, 1], F32)
    nc.vector.tensor_sub(soff[:], nwmax[:], lnwsum[:])

    bias05_t = coefpack[:, 0:K]
    nc.vector.tensor_scalar(bias05_t, mu, -1.0, 0.5,
                            op0=ALU.mult, op1=ALU.add)
    # c0 = w + soff - logsig - 0.5*log(2pi)
    c0_t = coefpack[:, 2 * K:3 * K]
    nc.vector.tensor_scalar(c0_t, w_, soff[:, 0:1], -LOG2PI_HALF,
                            op0=ALU.add, op1=ALU.add)
    nc.vector.tensor_sub(c0_t, c0_t, logsig[:])

    # on-chip permutation (BL, 3K) -> (128, 3) with partition p = b*K + k:
    # one matmul with eselT spreads row b to partition group b, then a
    # k-diagonal mask + reduce picks element k(p) per partition. No DMA, so
    # nothing here can queue behind the saturated memory stream.
    spread = ps_mlp.tile([128, 3 * K], F32, tag="mm1")
    nc.tensor.matmul(spread[:], eselT_sb[:], coefpack[:], start=True,
                     stop=True)
    picked = small.tile([128, 3 * K], F32)
    nc.vector.tensor_mul(picked[:], spread[:], dmask_sb[:])
    coefT = consts.tile([128, 3], F32)
    nc.vector.tensor_reduce(coefT[:], picked.rearrange("p (j k) -> p j k", k=K),
                            axis=mybir.AxisListType.X, op=ALU.add)

    # PE warm-up filler: dummy f32 matmuls in the PE-idle window between the
    # coefficient math and the score stage, so the HAM clock gate keeps the
    # PE at 2.4 GHz. Results are never read.
    warm = ps_mlp.tile([128, 512], F32, tag="mm1")
    for _ in range(8):
        nc.tensor.matmul(warm[:], w1_sb[:, 0:128], iota_sb[:, 0:512],
                         start=True, stop=True)
    bias05 = coefT[:, 0:1]   # 0.5 - mu
    nscale = coefT[:, 1:2]   # -1/(2 sigma^2)
    c0col = coefT[:, 2:3]    # logW - ln sigma - .5 ln 2pi

    # mask prep (off the critical path): mker = 1 - mask as f32
    mask_u8 = small.tile([BL, TT], U8)
    nc.scalar.dma_start(mask_u8[:], mask_d[:])
    mker = small.tile([BL, TT], F32, tag="mker")
    nc.vector.tensor_copy(mker[:], mask_u8[:])
    nc.vector.tensor_scalar(mker[:], mker[:], -1.0, 1.0,
                            op0=ALU.mult, op1=ALU.add)

    ident = consts.tile([BL, BL], F32)
    nc.gpsimd.memset(ident[:], 0.0)
    nc.gpsimd.affine_select(out=ident[:], in_=ident[:],
                            compare_op=ALU.not_equal, fill=1.0, base=0,
                            pattern=[[-1, BL]], channel_multiplier=1)

    # ---- stage 3: S[b, t] = sum_k exp(c0 - (J + .5 - mu)^2 / (2 s^2)) -----
    # The reference then does energy = softmax(where(mask, -1e8,
    # max(ln S, -18))). In the exp domain that whole chain is simply
    # S'' = max(S, e^-18) * (1 - mask); softmax needs no max-subtraction
    # because scores are bounded (<= ~3), so energy = S'' / sum(S'').
    # Everything below is per-512-chunk and pipelines across engines.
    # The energy transpose for the context matvec also happens per chunk on
    # unnormalized S'' (the 1/sum is folded into the PSUM drain later), so
    # the memory-streaming stage can start as early as possible.
    C18 = float(np.float32(np.exp(np.float32(-18.0))))
    S_sb = small.tile([BL, TT], F32)
    psums = small.tile([BL, NC512], F32)
    eT = consts.tile([128, 32 * BL], MEM_DT)
    S_r = S_sb.rearrange("b (G p r) -> b G r p", p=128, r=8)
    for cc in range(NC512):
        nc.tensor.matmul(warm[:], w1_sb[:, 0:128], iota_sb[:, 0:512],
                         start=True, stop=True)
        sq = ppool.tile([128, 512], F32, tag="sq")
        nc.scalar.activation(sq[:], iota_sb[:, bass.ts(cc, 512)], AF.Square,
                             bias=bias05)
        pt = ppool.tile([128, 512], F32, tag="pt")
        nc.scalar.activation(pt[:], sq[:], AF.Exp, bias=c0col, scale=nscale)
        sp = ps_s.tile([BL, 512], F32, tag="sp")
        nc.tensor.matmul(sp[:], esel_sb[:], pt[:], start=True, stop=True)
        scc = S_sb[:, bass.ts(cc, 512)]
        nc.vector.tensor_scalar_max(scc, sp[:], C18)
        nc.vector.tensor_mul(scc, scc, mker[:, bass.ts(cc, 512)])
        nc.vector.tensor_reduce(psums[:, cc:cc + 1], scc,
                                axis=mybir.AxisListType.X, op=ALU.add)

    # Transposes AFTER the full score loop: a (G, r) transpose reads columns
    # G*1024 + 8p + r, i.e. BOTH 512-chunks of block G -- emitting it inside
    # the chunk loop raced the second chunk's writes (reads of
    # later-in-program writes get no dependency).
    for G in range(NGRP):
        for r in range(8):
            tr = ps_tr.tile([128, BL], F32, tag="tr")
            nc.tensor.transpose(tr[:], S_r[:, G:G + 1, r:r + 1, :], ident[:])
            nc.vector.tensor_copy(eT[:, bass.ts(G * 8 + r, BL)], tr[:])

    # ---- stage 4: normalization ------------------------------------------
    tot = small.tile([BL, 1], F32)
    nc.vector.tensor_reduce(tot[:], psums[:], axis=mybir.AxisListType.X,
                            op=ALU.add)
    rsum = small.tile([BL, 1], F32)
    nc.vector.reciprocal(rsum[:], tot[:])
    # energy output (normalized); off the context critical path
    EN = small.tile([BL, TT], F32, tag="mker")
    nc.vector.tensor_scalar_mul(EN[:], S_sb[:], rsum[:, 0:1])
    nc.scalar.dma_start(energy_d[:], EN[:])
    # 1/sum as a row vector at partition 0 so the per-batch PSUM drain can
    # scale with an aligned (1,1) AP
    # reuse the long-dead mm2 PSUM slot to stay within the 8-bank budget
    trs = ps_mlp.tile([1, BL], F32, tag="mm1")
    nc.tensor.transpose(trs[:], rsum[:], ident[:])
    rs8 = small.tile([1, BL], F32)
    nc.vector.tensor_copy(rs8[:], trs[:])

    # ---- stage 5: ctx[b] = sum_t energy[b, t] * mem[b, t, :] --------------
    # mem tile (b, G): [p, r*F + f] = mem[b, G*1024 + 8p + r, f] -- 24 KB
    # contiguous per partition, one 3 MB DMA per tile.
    mem_r = mem_d.rearrange("b (G p r) f -> (b G) p (r f)", p=128, r=8)
    for b in range(BL):
        cab = ps_ctx.tile([1, F], F32, tag="cab")
        for g in range(NGRP):
            mt = mem_pool.tile([128, 8 * F], MEM_DT, tag="mt")
            nc.sync.dma_start(mt[:], mem_r[b * NGRP + g])
            if b < 3:
                # ramp-phase HAM filler: PE races the DMA tile-by-tile here
                # and 1-2us waits per tile cool the clock gate to 1.2 GHz;
                # one dummy matmul per group fills the wait and keeps the
                # PE at 2.4 GHz. Results are never read.
                nc.tensor.matmul(warm[:], w1_sb[:, 0:128], iota_sb[:, 0:512],
                                 start=True, stop=True)
            for r in range(8):
                lhs_col = (g * 8 + r) * BL + b
                lhs = eT[:, lhs_col:lhs_col + 1]
                rhsA = mt[:, r * F:r * F + 512]
                rhsB = mt[:, r * F + 512:(r + 1) * F]
                st = (g == 0 and r == 0)
                sp_ = (g == NGRP - 1 and r == 7)
                nc.tensor.matmul(cab[0:1, 0:512], lhs, rhsA,
                                 start=st, stop=sp_)
                nc.tensor.matmul(cab[0:1, 512:F], lhs, rhsB,
                                 start=st, stop=sp_)
        stage = small.tile([1, F], F32, tag="ctx_stage", bufs=2)
        nc.scalar.activation(stage[:], cab[:], AF.Copy,
                             scale=rs8[0:1, b:b + 1])
        nc.scalar.dma_start(ctx_d[b:b + 1, :], stage[:])


_compiled = None


def _get_compiled():
    global _compiled
    if _compiled is None:
        _compiled = _build_bass()
    return _compiled


def _host_prep(W1, b1, W2):
    W1a = np.ascontiguousarray(np.asarray(W1, np.float32)[:H])      # (H, A)
    # w1_sb[p, c*128 + a] = W1a[c*128 + p, a]
    w1_pre = np.ascontiguousarray(
        W1a.reshape(H // 128, 128, A).transpose(1, 0, 2).reshape(128, H)
    )
    b1_pre = np.ascontiguousarray(
        2.0 * np.asarray(b1, np.float32).reshape(A, 1)
    )
    w2_pre = np.ascontiguousarray(np.asarray(W2, np.float32))       # (A, 3K)
    esel = np.repeat(np.eye(BL, dtype=np.float32), K, axis=0)       # (128, BL)
    eselT = np.ascontiguousarray(esel.T)                             # (BL, 128)
    k16 = np.tile(np.eye(K, dtype=np.float32), (BL, 1))              # (128, K)
    dmask = np.concatenate([k16, k16, k16], axis=1)                  # (128, 3K)
    return w1_pre, b1_pre, w2_pre, esel, eselT, dmask


def kernel(attention_hidden_state, memory, processed_memory,
           attention_weights_cat, mask, W1, b1, W2):
    nc = _get_compiled()

    w1_pre, b1_pre, w2_pre, esel, eselT, dmask = _host_prep(W1, b1, W2)
    h = np.asarray(attention_hidden_state, np.float32)
    memory = np.asarray(memory, np.float32)
    if MEM_MODE == "bf16":
        memory = memory.astype(ml_dtypes.bfloat16)
    elif MEM_MODE == "f16":
        memory = memory.astype(np.float16)
    mask_u8 = np.asarray(mask).astype(np.uint8)

    in_maps = []
    for i in range(NCORES):
        hs = h[i * BL:(i + 1) * BL]                                  # (BL, H)
        # hT_pre[p, c*BL + b] = hs[b, c*128 + p]
        hT_pre = np.ascontiguousarray(
            hs.T.reshape(H // 128, 128, BL).transpose(1, 0, 2).reshape(128, 8 * BL)
        )
        in_maps.append({
            "hT": hT_pre,
            "mem": np.ascontiguousarray(memory[i * BL:(i + 1) * BL]),
            "mask": np.ascontiguousarray(mask_u8[i * BL:(i + 1) * BL]),
            "w1": w1_pre,
            "b1": b1_pre,
            "w2": w2_pre,
            "esel": esel,
            "eselT": eselT,
            "dmask": dmask,
        })

    res = run_bass_kernel_spmd(nc, in_maps, core_ids=list(range(NCORES)))
    next_ctx = np.concatenate([r["ctx"] for r in res.results], axis=0)
    energy = np.concatenate([r["energy"] for r in res.results], axis=0)
    return next_ctx, energy


# revision 34
# speedup vs baseline: 1.0602x; 1.0602x over previous
"""Gaussian-mixture attention (first decoder step) on 8 Trainium2 NeuronCores.

Data-parallel over the batch dim (64 -> 8 cores x 8). All math runs on device;
the host only shards/relayouts inputs and concatenates outputs.

Per-core pipeline:
  1. MLP: hiddenT = tanh(W1'^T @ h^T + b1)  (PE, contraction over H=1024)
     mlp = hiddenT^T @ W2 -> (8, 48)        (only first H rows of W1 matter:
     last context vector is exactly zero on the first step)
  2. coefficients per (b,k): mu = softplus(delta), Sigma = softplus(sigma),
     logW = log_softmax(w). Folded into ACT-friendly per-partition scalars
     (bias05 = 0.5 - mu, ninv2s = -1/(2 Sigma^2), c0 = logW - ln Sigma - .5*ln 2pi)
     and moved to the (b*16+k) partition layout via a tiny DRAM roundtrip.
  3. P[(b,k), t] = exp(c0 - (J_t - mu)^2 / (2 Sigma^2)) in two ACT passes over
     (128, 512) tiles (J from a one-time iota); sum over k via one PE matmul
     with a 0/1 block-selector (128, 8) -> S[b, t] in PSUM.
     log then clamp at -18: plain sum-exp is exact-equivalent to the
     reference's stable logsumexp because anything below -18 is clamped and
     values above -80 don't underflow.
  4. mask + softmax over t on (8, 4096) in place.
  5. energy chunks transposed on PE (interleaved with the score stage);
     context matvec ctx[b] = sum_t energy[b,t] * memory[b,t,:] accumulated
     in PSUM while 32 DMAs stream the memory shard (the roofline term).
     The 1/sum(energy) normalization is folded into the PSUM drain.

Precision: memory streams as fp16 by default (MEM_MODE env var overrides:
f32 / f32r / bf16). fp16 halves the HBM traffic and runs the PE at full
rate; measured output error is ~5e-4 scale-relative on ctx (vs ~8e-5 for
full f32, ~2e-4 for f32r) while the energy output stays at ~1e-5.
"""

import math
import os
from contextlib import ExitStack

import ml_dtypes
import numpy as np

import concourse.bacc as bacc
import concourse.bass as bass
import concourse.mybir as mybir
import concourse.tile as tile
from concourse.bass_utils import run_bass_kernel_spmd

AF = mybir.ActivationFunctionType
ALU = mybir.AluOpType
F32 = mybir.dt.float32
U8 = mybir.dt.uint8

B, TT, H, F, A, K = 64, 4096, 1024, 768, 128, 16
NCORES = 8
BL = B // NCORES          # 8 batches per core
NC512 = TT // 512         # 8 chunks for the score stage
NGRP = TT // 1024         # 4 memory tiles per batch (8 t-chunks of 128 each)

# context-matvec precision mode:
#   f32  -- exact fp32 matmuls (4 cycles/row on the PE)
#   f32r -- fp32 data in single-pass PE mode (1 cycle/row at N>=256)
#   bf16 -- memory downcast to bf16 on host (half DMA traffic, 1 cycle/row)
MEM_MODE = os.environ.get("NN_ATTN_MEM_MODE", "f16")
MEM_DT = {"bf16": mybir.dt.bfloat16,
          "f16": mybir.dt.float16,
          "f32r": mybir.dt.float32r,
          "f32": F32}[MEM_MODE]
# ~120 KB/partition of SBUF runway for the memory stream either way
MEM_BUFS = 10 if MEM_DT in (mybir.dt.bfloat16, mybir.dt.float16) else 5

LOG2PI_HALF = 0.5 * math.log(2.0 * math.pi)


def _build_bass():
    nc = bacc.Bacc(
        "TRN2", target_bir_lowering=False, debug=False, num_devices=NCORES
    )

    hT_d = nc.dram_tensor("hT", [128, 8 * BL], F32, kind="ExternalInput").ap()
    mem_d = nc.dram_tensor("mem", [BL, TT, F], MEM_DT, kind="ExternalInput").ap()
    mask_d = nc.dram_tensor("mask", [BL, TT], U8, kind="ExternalInput").ap()
    w1_d = nc.dram_tensor("w1", [128, H], F32, kind="ExternalInput").ap()
    b1_d = nc.dram_tensor("b1", [128, 1], F32, kind="ExternalInput").ap()
    w2_d = nc.dram_tensor("w2", [128, 3 * K], F32, kind="ExternalInput").ap()
    esel_d = nc.dram_tensor("esel", [128, BL], F32, kind="ExternalInput").ap()
    eselT_d = nc.dram_tensor("eselT", [BL, 128], F32, kind="ExternalInput").ap()
    dmask_d = nc.dram_tensor("dmask", [128, 3 * K], F32, kind="ExternalInput").ap()

    ctx_d = nc.dram_tensor("ctx", [BL, F], F32, kind="ExternalOutput").ap()
    energy_d = nc.dram_tensor("energy", [BL, TT], F32, kind="ExternalOutput").ap()

    with tile.TileContext(nc) as tc, ExitStack() as ctx:
        _kernel(ctx, tc, hT_d, mem_d, mask_d, w1_d, b1_d, w2_d, esel_d,
                eselT_d, dmask_d, ctx_d, energy_d)
    nc.compile()
    return nc


def _kernel(ctx, tc, hT_d, mem_d, mask_d, w1_d, b1_d, w2_d, esel_d,
            eselT_d, dmask_d, ctx_d, energy_d):
    nc = tc.nc

    consts = ctx.enter_context(tc.tile_pool(name="consts", bufs=1))
    small = ctx.enter_context(tc.tile_pool(name="small", bufs=1))
    ppool = ctx.enter_context(tc.tile_pool(name="ppool", bufs=3))
    mem_pool = ctx.enter_context(tc.tile_pool(name="mem_pool", bufs=MEM_BUFS))
    dram = ctx.enter_context(tc.tile_pool(name="dram", bufs=1, space="DRAM"))

    # PSUM bank budget (8 banks): mlp 2 (two tags x 1) + sp 3 + tr 1 +
    # ctx 2 (one (1,768) tile spanning 2 banks)
    ps_mlp = ctx.enter_context(tc.tile_pool(name="ps_mlp", bufs=1, space="PSUM"))
    ps_s = ctx.enter_context(tc.tile_pool(name="ps_s", bufs=1, space="PSUM"))
    ps_tr = ctx.enter_context(tc.tile_pool(name="ps_tr", bufs=2, space="PSUM"))
    ps_ctx = ctx.enter_context(tc.tile_pool(name="ps_ctx", bufs=2, space="PSUM"))

    # ---- constant / small input loads -------------------------------------
    w1_sb = consts.tile([128, H], F32)
    nc.scalar.dma_start(w1_sb[:], w1_d[:])
    w2_sb = consts.tile([128, 3 * K], F32)
    nc.scalar.dma_start(w2_sb[:], w2_d[:])
    b1_sb = consts.tile([128, 1], F32)
    nc.scalar.dma_start(b1_sb[:], b1_d[:])
    esel_sb = consts.tile([128, BL], F32)
    nc.scalar.dma_start(esel_sb[:], esel_d[:])
    eselT_sb = consts.tile([BL, 128], F32)
    nc.scalar.dma_start(eselT_sb[:], eselT_d[:])
    dmask_sb = consts.tile([128, 3 * K], F32)
    nc.scalar.dma_start(dmask_sb[:], dmask_d[:])
    hT_sb = consts.tile([128, 8 * BL], F32)
    nc.scalar.dma_start(hT_sb[:], hT_d[:])


    # J tile: value t at [p, t] for every partition (one-time iota, f32
    # exact; generated during the MLP head, off the critical path)
    iota_sb = consts.tile([128, TT], F32)
    nc.gpsimd.iota(iota_sb[:], pattern=[[1, TT]], base=0, channel_multiplier=0,
                   allow_small_or_imprecise_dtypes=True)

    # ---- stage 1: MLP ------------------------------------------------------
    mm1 = ps_mlp.tile([128, BL], F32, tag="mm1")
    for c in range(H // 128):
        nc.tensor.matmul(mm1[:], w1_sb[:, bass.ts(c, 128)],
                         hT_sb[:, bass.ts(c, BL)],
                         start=(c == 0), stop=(c == H // 128 - 1))
    # tanh(y) = 1 - 2/(exp(2y) + 1), y = mm1 + b1 (b1_sb holds 2*b1 so the
    # bias folds into the Exp's scale/bias form). Keeps every activation in
    # the single natural_log_exp table (Tanh would force a table reload).
    e2y = small.tile([128, BL], F32)
    nc.scalar.activation(e2y[:], mm1[:], AF.Exp, bias=b1_sb[:, 0:1], scale=2.0)
    nc.vector.tensor_scalar_add(e2y[:], e2y[:], 1.0)
    hidT = small.tile([128, BL], F32)
    nc.vector.reciprocal(hidT[:], e2y[:])
    nc.vector.tensor_scalar(hidT[:], hidT[:], -2.0, 1.0,
                            op0=ALU.mult, op1=ALU.add)

    mm2 = ps_mlp.tile([BL, 3 * K], F32, tag="mm1")
    nc.tensor.matmul(mm2[:], hidT[:], w2_sb[:], start=True, stop=True)
    mlp_sb = small.tile([BL, 3 * K], F32)
    nc.vector.tensor_copy(mlp_sb[:], mm2[:])

    # ---- stage 2: per-(b,k) coefficients in (BL, K) layout ----------------
    w_ = mlp_sb[:, 0:K]
    d_ = mlp_sb[:, K:2 * K]
    s_ = mlp_sb[:, 2 * K:3 * K]

    # softplus(x) = ln(1 + exp(x)) (Softplus has no ACT table on this build;
    # |x| <~ 5 here so the direct form is accurate in f32). d_ and s_ are
    # adjacent in mlp_sb, so both softplus chains run as one (8, 32) chain.
    ds = mlp_sb[:, K:3 * K]
    musig = small.tile([BL, 2 * K], F32)
    nc.scalar.activation(musig[:], ds, AF.Exp)
    nc.vector.tensor_scalar_add(musig[:], musig[:], 1.0)
    nc.scalar.activation(musig[:], musig[:], AF.Ln)
    mu = musig[:, 0:K]
    sig = musig[:, K:2 * K]

    coefpack = small.tile([BL, 3 * K], F32)
    # -1 / (2 sigma^2)
    sig2 = small.tile([BL, K], F32)
    nc.vector.tensor_mul(sig2[:], sig, sig)
    nc.vector.tensor_scalar_mul(sig2[:], sig2[:], 2.0)
    ninv2s = coefpack[:, K:2 * K]
    nc.vector.reciprocal(ninv2s, sig2[:])
    nc.vector.tensor_scalar_mul(ninv2s, ninv2s, -1.0)

    logsig = small.tile([BL, K], F32)
    nc.scalar.activation(logsig[:], sig, AF.Ln)

    # log-softmax of w over k (free dim)
    nwmax = small.tile([BL, 1], F32)
    nc.vector.tensor_reduce(nwmax[:], w_, axis=mybir.AxisListType.X,
                            op=ALU.max, negate=True)
    expw = small.tile([BL, K], F32)
    wsum = small.tile([BL, 1], F32)
    nc.scalar.activation(expw[:], w_, AF.Exp, bias=nwmax[:, 0:1],
                         accum_out=wsum[:, 0:1])
    lnwsum = small.tile([BL, 1], F32)
    nc.scalar.activation(lnwsum[:], wsum[:], AF.Ln)
    # soff = -max - ln(sum)
    soff = small.tile([BL, 1], F32)
    nc.vector.tensor_sub(soff[:], nwmax[:], lnwsum[:])

    bias05_t = coefpack[:, 0:K]
    nc.vector.tensor_scalar(bias05_t, mu, -1.0, 0.5,
                            op0=ALU.mult, op1=ALU.add)
    # c0 = w + soff - logsig - 0.5*log(2pi)
    c0_t = coefpack[:, 2 * K:3 * K]
    nc.vector.tensor_scalar(c0_t, w_, soff[:, 0:1], -LOG2PI_HALF,
                            op0=ALU.add, op1=ALU.add)
    nc.vector.tensor_sub(c0_t, c0_t, logsig[:])

    # on-chip permutation (BL, 3K) -> (128, 3) with partition p = b*K + k:
    # one matmul with eselT spreads row b to partition group b, then a
    # k-diagonal mask + reduce picks element k(p) per partition. No DMA, so
    # nothing here can queue behind the saturated memory stream.
    spread = ps_mlp.tile([128, 3 * K], F32, tag="mm1")
    nc.tensor.matmul(spread[:], eselT_sb[:], coefpack[:], start=True,
                     stop=True)
    picked = small.tile([128, 3 * K], F32)
    nc.vector.tensor_mul(picked[:], spread[:], dmask_sb[:])
    coefT = consts.tile([128, 3], F32)
    nc.vector.tensor_reduce(coefT[:], picked.rearrange("p (j k) -> p j k", k=K),
                            axis=mybir.AxisListType.X, op=ALU.add)

    # PE warm-up filler: dummy f32 matmuls in the PE-idle window between the
    # coefficient math and the score stage, so the HAM clock gate keeps the
    # PE at 2.4 GHz. Results are never read.
    warm = ps_mlp.tile([128, 512], F32, tag="mm1")
    for _ in range(8):
        nc.tensor.matmul(warm[:], w1_sb[:, 0:128], iota_sb[:, 0:512],
                         start=True, stop=True)
    bias05 = coefT[:, 0:1]   # 0.5 - mu
    nscale = coefT[:, 1:2]   # -1/(2 sigma^2)
    c0col = coefT[:, 2:3]    # logW - ln sigma - .5 ln 2pi

    # mask prep (off the critical path): mker = 1 - mask as f32
    mask_u8 = small.tile([BL, TT], U8)
    nc.scalar.dma_start(mask_u8[:], mask_d[:])
    mker = small.tile([BL, TT], F32, tag="mker")
    nc.vector.tensor_copy(mker[:], mask_u8[:])
    nc.vector.tensor_scalar(mker[:], mker[:], -1.0, 1.0,
                            op0=ALU.mult, op1=ALU.add)

    ident = consts.tile([BL, BL], F32)
    nc.gpsimd.memset(ident[:], 0.0)
    nc.gpsimd.affine_select(out=ident[:], in_=ident[:],
                            compare_op=ALU.not_equal, fill=1.0, base=0,
                            pattern=[[-1, BL]], channel_multiplier=1)

    # ---- stage 3: S[b, t] = sum_k exp(c0 - (J + .5 - mu)^2 / (2 s^2)) -----
    # The reference then does energy = softmax(where(mask, -1e8,
    # max(ln S, -18))). In the exp domain that whole chain is simply
    # S'' = max(S, e^-18) * (1 - mask); softmax needs no max-subtraction
    # because scores are bounded (<= ~3), so energy = S'' / sum(S'').
    # Everything below is per-512-chunk and pipelines across engines.
    # The energy transpose for the context matvec also happens per chunk on
    # unnormalized S'' (the 1/sum is folded into the PSUM drain later), so
    # the memory-streaming stage can start as early as possible.
    C18 = float(np.float32(np.exp(np.float32(-18.0))))
    S_sb = small.tile([BL, TT], F32)
    psums = small.tile([BL, NC512], F32)
    eT = consts.tile([128, 32 * BL], MEM_DT)
    S_r = S_sb.rearrange("b (G p r) -> b G r p", p=128, r=8)
    for cc in range(NC512):
        nc.tensor.matmul(warm[:], w1_sb[:, 0:128], iota_sb[:, 0:512],
                         start=True, stop=True)
        sq = ppool.tile([128, 512], F32, tag="sq")
        nc.scalar.activation(sq[:], iota_sb[:, bass.ts(cc, 512)], AF.Square,
                             bias=bias05)
        pt = ppool.tile([128, 512], F32, tag="pt")
        nc.scalar.activation(pt[:], sq[:], AF.Exp, bias=c0col, scale=nscale)
        sp = ps_s.tile([BL, 512], F32, tag="sp")
        nc.tensor.matmul(sp[:], esel_sb[:], pt[:], start=True, stop=True)
        scc = S_sb[:, bass.ts(cc, 512)]
        nc.vector.tensor_scalar_max(scc, sp[:], C18)
        nc.vector.tensor_mul(scc, scc, mker[:, bass.ts(cc, 512)])
        nc.vector.tensor_reduce(psums[:, cc:cc + 1], scc,
                                axis=mybir.AxisListType.X, op=ALU.add)

    # Transposes AFTER the full score loop: a (G, r) transpose reads columns
    # G*1024 + 8p + r, i.e. BOTH 512-chunks of block G -- emitting it inside
    # the chunk loop raced the second chunk's writes (reads of
    # later-in-program writes get no dependency).
    for G in range(NGRP):
        for r in range(8):
            tr = ps_tr.tile([128, BL], F32, tag="tr")
            nc.tensor.transpose(tr[:], S_r[:, G:G + 1, r:r + 1, :], ident[:])
            nc.vector.tensor_copy(eT[:, bass.ts(G * 8 + r, BL)], tr[:])

    # ---- stage 4: normalization ------------------------------------------
    tot = small.tile([BL, 1], F32)
    nc.vector.tensor_reduce(tot[:], psums[:], axis=mybir.AxisListType.X,
                            op=ALU.add)
    rsum = small.tile([BL, 1], F32)
    nc.vector.reciprocal(rsum[:], tot[:])
    # energy output (normalized); off the context critical path
    EN = small.tile([BL, TT], F32, tag="mker")
    nc.vector.tensor_scalar_mul(EN[:], S_sb[:], rsum[:, 0:1])
    nc.scalar.dma_start(energy_d[:], EN[:])
    # 1/sum as a row vector at partition 0 so the per-batch PSUM drain can
    # scale with an aligned (1,1) AP
    # reuse the long-dead mm2 PSUM slot to stay within the 8-bank budget
    trs = ps_mlp.tile([1, BL], F32, tag="mm1")
    nc.tensor.transpose(trs[:], rsum[:], ident[:])
    rs8 = small.tile([1, BL], F32)
    nc.vector.tensor_copy(rs8[:], trs[:])

    # ---- stage 5: ctx[b] = sum_t energy[b, t] * mem[b, t, :] --------------
    # mem tile (b, G): [p, r*F + f] = mem[b, G*1024 + 8p + r, f] -- 24 KB
    # contiguous per partition, one 3 MB DMA per tile.
    mem_r = mem_d.rearrange("b (G p r) f -> (b G) p (r f)", p=128, r=8)
    for b in range(BL):
        cab = ps_ctx.tile([1, F], F32, tag="cab")
        for g in range(NGRP):
            mt = mem_pool.tile([128, 8 * F], MEM_DT, tag="mt")
            nc.sync.dma_start(mt[:], mem_r[b * NGRP + g])
            for r in range(8):
                lhs_col = (g * 8 + r) * BL + b
                lhs = eT[:, lhs_col:lhs_col + 1]
                rhsA = mt[:, r * F:r * F + 512]
                rhsB = mt[:, r * F + 512:(r + 1) * F]
                st = (g == 0 and r == 0)
                sp_ = (g == NGRP - 1 and r == 7)
                nc.tensor.matmul(cab[0:1, 0:512], lhs, rhsA,
                                 start=st, stop=sp_)
                nc.tensor.matmul(cab[0:1, 512:F], lhs, rhsB,
                                 start=st, stop=sp_)
        stage = small.tile([1, F], F32, tag="ctx_stage", bufs=2)
        nc.scalar.activation(stage[:], cab[:], AF.Copy,
                             scale=rs8[0:1, b:b + 1])
        nc.scalar.dma_start(ctx_d[b:b + 1, :], stage[:])


_compiled = None


def _get_compiled():
    global _compiled
    if _compiled is None:
        _compiled = _build_bass()
    return _compiled


def _host_prep(W1, b1, W2):
    W1a = np.ascontiguousarray(np.asarray(W1, np.float32)[:H])      # (H, A)
    # w1_sb[p, c*128 + a] = W1a[c*128 + p, a]
    w1_pre = np.ascontiguousarray(
        W1a.reshape(H // 128, 128, A).transpose(1, 0, 2).reshape(128, H)
    )
    b1_pre = np.ascontiguousarray(
        2.0 * np.asarray(b1, np.float32).reshape(A, 1)
    )
    w2_pre = np.ascontiguousarray(np.asarray(W2, np.float32))       # (A, 3K)
    esel = np.repeat(np.eye(BL, dtype=np.float32), K, axis=0)       # (128, BL)
    eselT = np.ascontiguousarray(esel.T)                             # (BL, 128)
    k16 = np.tile(np.eye(K, dtype=np.float32), (BL, 1))              # (128, K)
    dmask = np.concatenate([k16, k16, k16], axis=1)                  # (128, 3K)
    return w1_pre, b1_pre, w2_pre, esel, eselT, dmask


def kernel(attention_hidden_state, memory, processed_memory,
           attention_weights_cat, mask, W1, b1, W2):
    nc = _get_compiled()

    w1_pre, b1_pre, w2_pre, esel, eselT, dmask = _host_prep(W1, b1, W2)
    h = np.asarray(attention_hidden_state, np.float32)
    memory = np.asarray(memory, np.float32)
    if MEM_MODE == "bf16":
        memory = memory.astype(ml_dtypes.bfloat16)
    elif MEM_MODE == "f16":
        memory = memory.astype(np.float16)
    mask_u8 = np.asarray(mask).astype(np.uint8)

    in_maps = []
    for i in range(NCORES):
        hs = h[i * BL:(i + 1) * BL]                                  # (BL, H)
        # hT_pre[p, c*BL + b] = hs[b, c*128 + p]
        hT_pre = np.ascontiguousarray(
            hs.T.reshape(H // 128, 128, BL).transpose(1, 0, 2).reshape(128, 8 * BL)
        )
        in_maps.append({
            "hT": hT_pre,
            "mem": np.ascontiguousarray(memory[i * BL:(i + 1) * BL]),
            "mask": np.ascontiguousarray(mask_u8[i * BL:(i + 1) * BL]),
            "w1": w1_pre,
            "b1": b1_pre,
            "w2": w2_pre,
            "esel": esel,
            "eselT": eselT,
            "dmask": dmask,
        })

    res = run_bass_kernel_spmd(nc, in_maps, core_ids=list(range(NCORES)))
    next_ctx = np.concatenate([r["ctx"] for r in res.results], axis=0)
    energy = np.concatenate([r["energy"] for r in res.results], axis=0)
    return next_ctx, energy
